# revision 3
# baseline (speedup 1.0000x reference)
"""ANFIS Trainium2 kernel (8 NeuronCores, Bass/Tile) — v5.

Math (reference):
  mfs[b,i,j] = exp(-(x[b,i]-centers[i,j])^2 / (2*widths[i,j]^2))   [1024,8,4]
  w[b,r]     = prod_i mfs[b,i,idx_i(r)]    r in [0, 4^8=65536), i0 slowest
  w        <- w / sum_r w
  out[b,n]   = sum_r w[b,r] * ([x[b],1] . rule_params[r,:,n])      [1024,16]

Structure: w = wA (x) wB with wA over dims 0..2 (64 vals, split 8 rA per
core) and wB over dims 3..7 (1024 vals); r = rA*1024 + rB.  Denominator
factorizes: sum_r w = prod_i (sum_j mfs[b,i,j]).

Per core:  psum[b, rA, i*16+n] = sum_rB wB[b,rB] rp[rA*1024+rB, i*16+n]
(bf16 matmuls, rB contracted on partitions, kt = 8 k-tiles), evacuated as
psum * G with G[b, rA*9+i] = wA[b,rA]/denom[b] * xb[b,i], tree-summed over
rA and strided-reduced over i.  Core partials summed on host.

v5 schedule notes (over v4 @58.1us):
  - rp is 8 per-kt DRAM params/tiles spread round-robin over the three
    DMA queues by first-use time: sync kt0/3/6, scalar kt1/4, gpsimd
    kt2/5/7.  Head inputs split: xabd (x only) FIRST on sync, cbw
    (centers+1/2w^2) + small2 (wA-chain consts) + eye on scalar.
    xA3 is an AP view of xabd now (was 98KB of small2).
  - j-scales fused: wq56[j] = w56 * mfs7[j] (4 tiny ACT muls) and ONE
    [128,1024] DVE TT w3s = w34 x wq56 replaces w3456 + 4x256 j-scales.
  - bt0 AND bt1 wB^T via PE identity-matmul transposes (XBAR trigger
    latency dodged); bt1's transposes interleave after bt0's kt3 where
    the PE would stall on rp DMA anyway.  XBARs bt2/4/6 on sync,
    bt3/5/7 on scalar, emitted after the head compute.
  - mains for bt2..7 run group-outer (g0 kt0..7, g1, g2) so each psum
    group closes 1/3 into the bt and evac overlaps the same bt's
    stream; bt0/bt1 stay kt-outer (DMA-paced).
  - evac tree adds on Pool (gpsimd) for bt>=1 (idle after its rp DMAs);
    DVE keeps the psum-reading xsc mults + reduce + scale.  Pair tree
    ordered so only ~3 Pool adds + reduce + scale trail the last matmul.
"""

import sys

sys.path.insert(0, "/opt/trn_rl_repo")

import numpy as np

import concourse.bacc as bacc
import concourse.tile as tile
import concourse.mybir as mybir
from concourse.ap import AP
from concourse.bass_utils import run_bass_kernel_spmd


F32 = mybir.dt.float32
BF16 = mybir.dt.bfloat16
MULT = mybir.AluOpType.mult
ADD = mybir.AluOpType.add
SUB = mybir.AluOpType.subtract
EXP = mybir.ActivationFunctionType.Exp
AXX = mybir.AxisListType.X

N_CORES = 8
B = 1024
BT = 8          # batch tiles of 128
D = 8           # input dims
DX = D + 1      # xb width (x plus ones column)
M = 4           # membership fns per dim
NO = 16         # outputs
C = DX * NO                 # 144
NRA = 64        # 4^3 (dims 0..2)
RA_LOC = NRA // N_CORES     # 8 local rA per core
NRB = 1024      # 4^5 (dims 3..7)
KT = 8          # rB partition tiles of 128
GROUPS = [(0, 3), (3, 3), (6, 2)]
SC = RA_LOC * C  # 1152
DM = D * M       # 32

N_WARM = 10             # dummy warm-up matmuls (256 cols each)

O_CB = 0
O_CW2N = O_CB + DM                # 32
NCBW = O_CW2N + DM                # 64
O_CA3 = 0
O_NWA2 = O_CA3 + RA_LOC * 3       # 24
NSM2 = O_NWA2 + RA_LOC * 3        # 48

# rp kt -> issuing engine queue (round-robin by first-use time)
RP_Q = {0: "sync", 3: "sync", 6: "sync",
        1: "scalar", 4: "scalar",
        2: "gpsimd", 5: "gpsimd", 7: "gpsimd"}
XBAR_Q = {2: "sync", 4: "sync", 6: "sync",
          3: "scalar", 5: "scalar", 7: "scalar"}


def _v(t, off, dims):
    """Custom free-dim view of a [128, F] SBUF tile AP."""
    part = list(t.ap[0])
    return AP(
        tensor=t.tensor,
        offset=t.offset + off,
        ap=[part] + [[s, n] for (s, n) in dims],
    )


def build_nc():
    nc = bacc.Bacc("TRN2", target_bir_lowering=False, debug=False,
                   num_devices=N_CORES)

    xabd_d = nc.declare_dram_parameter("xabd", [128, BT * DX], F32,
                                       isOutput=False)
    cbw_d = nc.declare_dram_parameter("cbw", [128, NCBW], F32,
                                      isOutput=False)
    small2_d = nc.declare_dram_parameter("small2", [128, NSM2], F32,
                                         isOutput=False)
    eye_d = nc.declare_dram_parameter("eye", [128, 128], BF16, isOutput=False)
    rp_d = [nc.declare_dram_parameter(f"rp{kt}", [128, SC], BF16,
                                      isOutput=False) for kt in range(KT)]
    out_d = nc.declare_dram_parameter("out", [B, NO], F32, isOutput=True)

    with tile.TileContext(nc) as tc:
        with (
            tc.tile_pool(name="const", bufs=1) as cpool,
            tc.tile_pool(name="rp", bufs=1) as rppool,
            tc.tile_pool(name="wbt", bufs=1) as wbtpool,
            tc.tile_pool(name="work", bufs=2) as work,
            tc.tile_pool(name="w3s", bufs=3) as w3spool,
            tc.tile_pool(name="psD", bufs=1, space="PSUM") as psDp,
            tc.tile_pool(name="evac", bufs=3) as evpool,
            tc.tile_pool(name="ps0", bufs=2, space="PSUM") as ps0p,
            tc.tile_pool(name="ps1", bufs=2, space="PSUM") as ps1p,
            tc.tile_pool(name="ps2", bufs=2, space="PSUM") as ps2p,
        ):
            # ---- input tiles ----
            xab_t = cpool.tile([128, BT * DX], F32, tag="xabd")
            cbw = cpool.tile([128, NCBW], F32, tag="cbw")
            small2 = cpool.tile([128, NSM2], F32, tag="small2")
            eye = cpool.tile([128, 128], BF16, tag="eye")
            rp = [rppool.tile([128, SC], BF16, tag=f"rp{kt}",
                              name=f"rp{kt}")
                  for kt in range(KT)]
            zs = cpool.tile([128, 512], BF16, tag="zs")

            engs = {"sync": nc.sync, "scalar": nc.scalar,
                    "gpsimd": nc.gpsimd}

            # DMA issue order per queue == emission order per engine.
            nc.sync.dma_start(xab_t[:], xabd_d[:])
            nc.scalar.dma_start(cbw[:], cbw_d[:])
            nc.scalar.dma_start(small2[:], small2_d[:])
            nc.scalar.dma_start(eye[:], eye_d[:])
            for kt in (0, 3, 6, 1, 4, 2, 5, 7):
                engs[RP_Q[kt]].dma_start(rp[kt][:], rp_d[kt][:])

            xab = xab_t[:]
            cb = cbw[:, O_CB:O_CB + DM]
            cw2n = cbw[:, O_CW2N:O_CW2N + DM]
            cA3 = small2[:, O_CA3:O_CA3 + RA_LOC * 3]
            nwA2 = small2[:, O_NWA2:O_NWA2 + RA_LOC * 3]

            # ---- PE warm-up: zero tile (DVE memset, no deps) + dummies ----
            nc.vector.memset(zs[:], 0)
            psD = [psDp.tile([128, 512], F32, tag="psD0", name="psD0"),
                   psDp.tile([128, 512], F32, tag="psD1", name="psD1")]
            for i in range(N_WARM):
                nc.tensor.matmul(psD[i % 2][:, 0:256], zs[:, 0:128],
                                 zs[:, 0:256], start=True, stop=True)

            # DVE / Pool stage chains: keep scheduler in emission order
            last_dve = [None]

            def dve(op_fn, *args, **kwargs):
                i = op_fn(*args, **kwargs)
                if last_dve[0] is not None:
                    tile.add_dep_helper(i.ins, last_dve[0].ins, sync=False,
                                        reason="dve stage order")
                last_dve[0] = i
                return i

            last_pool = [None]

            def pool(op_fn, *args, **kwargs):
                i = op_fn(*args, **kwargs)
                if last_pool[0] is not None:
                    tile.add_dep_helper(i.ins, last_pool[0].ins, sync=False,
                                        reason="pool stage order")
                last_pool[0] = i
                return i

            # bt0 membership chain in its own small tiles (clean DMA dep)
            mfs0 = cpool.tile([128, DM], F32, tag="mfs0")
            mfsR = cpool.tile([128, (BT - 1) * DM], F32, tag="mfsR")

            def mfs_chain(mfst, nbt, xoff, tg):
                dift = work.tile([128, nbt * DM], F32, tag="dif" + tg)
                d2t = work.tile([128, nbt * DM], F32, tag="d2" + tg)
                d2st = work.tile([128, nbt * DM], F32, tag="d2s" + tg)
                dve(nc.vector.tensor_tensor,
                    _v(dift[:], 0, [(DM, nbt), (M, D), (1, M)]),
                    _v(xab, xoff, [(DX, nbt), (1, D), (0, M)]),
                    _v(cb, 0, [(0, nbt), (M, D), (1, M)]),
                    op=SUB)
                dve(nc.vector.tensor_tensor,
                    d2t[:], dift[:], dift[:], op=MULT)
                dve(nc.vector.tensor_tensor,
                    _v(d2st[:], 0, [(DM, nbt), (1, DM)]),
                    _v(d2t[:], 0, [(DM, nbt), (1, DM)]),
                    _v(cw2n, 0, [(0, nbt), (1, DM)]), op=MULT)
                nc.scalar.activation(mfst[:], d2st[:], EXP, scale=-1.0)

            # mfs column views per bt (bt0 in mfs0, rest in mfsR)
            def mfs_at(bt, off):
                if bt == 0:
                    return mfs0, off
                return mfsR, (bt - 1) * DM + off

            w34 = work.tile([128, BT * 16], BF16, tag="w34")
            w56 = work.tile([128, BT * 16], BF16, tag="w56")
            wq56 = cpool.tile([128, BT * 64], BF16, tag="wq56")

            def w_chain(off, nbt):
                mfst, moff = mfs_at(off, 0)
                dve(nc.vector.tensor_tensor,
                    _v(w34[:], off * 16, [(16, nbt), (M, M), (1, M)]),
                    _v(mfst[:], moff + 3 * M, [(DM, nbt), (1, M), (0, M)]),
                    _v(mfst[:], moff + 4 * M, [(DM, nbt), (0, M), (1, M)]),
                    op=MULT)
                dve(nc.vector.tensor_tensor,
                    _v(w56[:], off * 16, [(16, nbt), (M, M), (1, M)]),
                    _v(mfst[:], moff + 5 * M, [(DM, nbt), (1, M), (0, M)]),
                    _v(mfst[:], moff + 6 * M, [(DM, nbt), (0, M), (1, M)]),
                    op=MULT)

            wbt = wbtpool.tile([128, KT * B], BF16, tag="wbt")

            def w3s_make(bt):
                """w3s[b, j*256+q*16+s] = w34[q]*w56[s]*mfs7[j].

                4 tiny ACT scalar-muls (wq56) + one [128,1024] DVE TT.
                """
                mfst, moff = mfs_at(bt, 7 * M)
                for j in range(M):
                    nc.scalar.mul(
                        wq56[:, bt * 64 + j * 16: bt * 64 + (j + 1) * 16],
                        w56[:, bt * 16:(bt + 1) * 16],
                        mfst[:, moff + j: moff + j + 1])
                w3sall = w3spool.tile([128, 1024], BF16, tag="w3s",
                                      name="w3sall")
                dve(nc.vector.tensor_tensor,
                    _v(w3sall[:], 0, [(256, M), (16, 16), (1, 16)]),
                    _v(w34[:], bt * 16, [(0, M), (1, 16), (0, 16)]),
                    _v(wq56[:], bt * 64, [(16, M), (0, 16), (1, 16)]),
                    op=MULT)
                return w3sall

            def pe_transpose(w3sb, bt, psDt):
                """wB^T for bt via PE identity matmuls + ACT psum copies."""
                for j in range(M):
                    for qh in range(2):
                        kt = 2 * j + qh
                        m, t = kt // 4, kt % 4
                        nc.tensor.matmul(
                            psDt[m][:, t * 128:(t + 1) * 128],
                            w3sb[:, kt * 128:(kt + 1) * 128], eye[:],
                            start=True, stop=True)
                # split so kt0's slice lands first
                nc.scalar.copy(_v(wbt[:], bt * 128, [(B, 1), (1, 128)]),
                               psDt[0][:, 0:128])
                nc.scalar.copy(_v(wbt[:], B + bt * 128, [(B, 3), (1, 128)]),
                               psDt[0][:, 128:512])
                nc.scalar.copy(_v(wbt[:], 4 * B + bt * 128,
                                  [(B, 4), (1, 128)]),
                               psDt[1][:])

            # ---- matmul emit helpers ----
            def mm(ps, bt, kt, g, start, stop):
                r0, nr = GROUPS[g]
                nc.tensor.matmul(
                    ps[g][:],
                    wbt[:, kt * B + bt * 128: kt * B + (bt + 1) * 128],
                    _v(rp[kt][:], r0 * C, [(C, nr), (1, C)]),
                    start=start, stop=stop)

            def alloc_ps():
                return [
                    ps0p.tile([128, GROUPS[0][1] * C], F32, tag="ps0",
                              name="ps0"),
                    ps1p.tile([128, GROUPS[1][1] * C], F32, tag="ps1",
                              name="ps1"),
                    ps2p.tile([128, GROUPS[2][1] * C], F32, tag="ps2",
                              name="ps2")]

            # ---- S1: bt0 membership chain FIRST, then bulk ----
            mfs_chain(mfs0, 1, 0, "0")
            w_chain(0, 1)
            w3s0 = w3s_make(0)
            mfs_chain(mfsR, BT - 1, DX, "R")

            pe_transpose(w3s0, 0, psD)

            # ---- bt0 mains (kt-outer, DMA-paced); bt1 PE transposes
            #      interleave after kt3 where the PE waits on rp anyway ----
            ps_bt = [None] * BT
            ps_bt[0] = alloc_ps()

            w_chain(1, 1)
            w3s1 = w3s_make(1)

            for kt in range(4):
                for g in range(3):
                    mm(ps_bt[0], 0, kt, g, start=(kt == 0), stop=False)
            pe_transpose(w3s1, 1, psD)
            for kt in range(4, KT):
                for g in range(3):
                    mm(ps_bt[0], 0, kt, g, start=False, stop=(kt == KT - 1))

            # ---- remaining w-chains + w3s + XBARs (head-emitted) ----
            w_chain(2, BT - 2)
            w3sb = {}
            for bt in range(2, BT):
                w3sb[bt] = w3s_make(bt)

            # ---- S4: wA chain -> G = wA*xb (UNNORMALIZED; 1/denom is
            #      applied per-bt to the reduced [128,16] output) ----
            NA = BT * RA_LOC * 3  # 192
            xA3v = _v(xab, 0, [(DX, BT), (0, RA_LOC), (1, 3)])
            dA = work.tile([128, NA], F32, tag="dA")
            dve(nc.vector.tensor_tensor,
                dA[:], xA3v, _v(cA3, 0, [(0, BT), (1, RA_LOC * 3)]), op=SUB)
            d2A = work.tile([128, NA], F32, tag="d2A")
            dve(nc.vector.tensor_tensor, d2A[:], dA[:], dA[:], op=MULT)
            d2sA = work.tile([128, NA], F32, tag="d2sA")
            dve(nc.vector.tensor_tensor,
                d2sA[:], d2A[:], _v(nwA2, 0, [(0, BT), (1, RA_LOC * 3)]),
                op=MULT)
            eA = work.tile([128, BT * RA_LOC], F32, tag="eA")
            dve(nc.vector.reduce_sum,
                eA[:], _v(d2sA[:], 0, [(3, BT * RA_LOC), (1, 3)]), axis=AXX)
            wA = cpool.tile([128, BT * RA_LOC], F32, tag="wA")
            nc.scalar.activation(wA[:], eA[:], EXP, scale=-1.0)

            Gall = cpool.tile([128, BT * RA_LOC * DX], F32, tag="Gall")
            dve(nc.vector.tensor_tensor,
                Gall[:],
                _v(wA[:], 0, [(RA_LOC, BT), (1, RA_LOC), (0, DX)]),
                _v(xab, 0, [(DX, BT), (0, RA_LOC), (1, DX)]), op=MULT)

            invd = cpool.tile([128, BT], F32, tag="invd")

            def denoms():
                s = work.tile([128, BT * D], F32, tag="s")
                dve(nc.vector.reduce_sum,
                    s[:, 0:D], _v(mfs0[:], 0, [(M, D), (1, M)]), axis=AXX)
                dve(nc.vector.reduce_sum,
                    s[:, D:BT * D],
                    _v(mfsR[:], 0, [(M, (BT - 1) * D), (1, M)]), axis=AXX)
                p1 = work.tile([128, BT * 4], F32, tag="p1")
                dve(nc.vector.tensor_tensor,
                    p1[:], _v(s[:], 0, [(D, BT), (1, 4)]),
                    _v(s[:], 4, [(D, BT), (1, 4)]), op=MULT)
                p2 = work.tile([128, BT * 2], F32, tag="p2")
                dve(nc.vector.tensor_tensor,
                    p2[:], _v(p1[:], 0, [(4, BT), (1, 2)]),
                    _v(p1[:], 2, [(4, BT), (1, 2)]), op=MULT)
                p3 = work.tile([128, BT], F32, tag="p3")
                dve(nc.vector.tensor_tensor,
                    p3[:], _v(p2[:], 0, [(2, BT)]), _v(p2[:], 1, [(2, BT)]),
                    op=MULT)
                dve(nc.vector.reciprocal, invd[:], p3[:])

            denoms()

            # XBARs bt2..7: sync gets 2/4/6, scalar 3/5/7 (scalar ones
            # AFTER all head ACT compute so their data-waits block nothing)
            for bt in (2, 4, 6):
                nc.sync.dma_start_transpose(
                    _v(wbt[:], bt * 128, [(B, KT), (1, 128)]), w3sb[bt][:])
            for bt in (3, 5, 7):
                nc.scalar.dma_start_transpose(
                    _v(wbt[:], bt * 128, [(B, KT), (1, 128)]), w3sb[bt][:])

            # ---- evac pieces ----
            def evac_mults_g(bt, ps, g, xsc):
                r0, nr = GROUPS[g]
                dve(nc.vector.tensor_tensor,
                    xsc[:, r0 * C:(r0 + nr) * C], ps[g][:],
                    _v(Gall[:], bt * RA_LOC * DX + r0 * DX,
                       [(DX, nr), (1, DX), (0, NO)]),
                    op=MULT)

            def evac_finish(bt, th3):
                ob = evpool.tile([128, NO], F32, tag="ob")
                dve(nc.vector.reduce_sum,
                    ob[:], _v(th3[:], 0, [(1, NO), (NO, DX)]), axis=AXX)
                obn = evpool.tile([128, NO], F32, tag="obn")
                dve(nc.vector.tensor_scalar_mul,
                    obn[:], ob[:], invd[:, bt:bt + 1])
                return obn

            def evac_dve_tree(bt, ps):
                """bt0 path: xsc all groups + full DVE tree."""
                xsc = evpool.tile([128, SC], BF16, tag="xsc")
                for g in range(3):
                    evac_mults_g(bt, ps, g, xsc)
                th = evpool.tile([128, 4 * C], BF16, tag="th")
                dve(nc.vector.tensor_tensor,
                    th[:], xsc[:, 0:4 * C], xsc[:, 4 * C:8 * C], op=ADD)
                th2 = evpool.tile([128, 2 * C], BF16, tag="th2")
                dve(nc.vector.tensor_tensor,
                    th2[:], th[:, 0:2 * C], th[:, 2 * C:4 * C], op=ADD)
                th3 = evpool.tile([128, C], BF16, tag="th3")
                dve(nc.vector.tensor_tensor,
                    th3[:], th2[:, 0:C], th2[:, C:2 * C], op=ADD)
                return evac_finish(bt, th3)

            def evac_pool_tree(bt, ps):
                """bt>=1: DVE xsc per group as it closes; Pool pair-tree.

                Pairing: q0=x0+x1 (in g0), q1=x2+x3, q2=x4+x5 (need g1),
                q3=x6+x7 (g2), h0=q0+q1, h1=q2+q3, th3=h0+h1.
                """
                xsc = evpool.tile([128, SC], BF16, tag="xsc")
                q = evpool.tile([128, 4 * C], BF16, tag="th")
                h = evpool.tile([128, 2 * C], BF16, tag="th2")
                th3 = evpool.tile([128, C], BF16, tag="th3")
                evac_mults_g(bt, ps, 0, xsc)
                pool(nc.gpsimd.tensor_tensor,
                     q[:, 0:C], xsc[:, 0:C], xsc[:, C:2 * C], op=ADD)
                evac_mults_g(bt, ps, 1, xsc)
                pool(nc.gpsimd.tensor_tensor,
                     q[:, C:2 * C], xsc[:, 2 * C:3 * C], xsc[:, 3 * C:4 * C],
                     op=ADD)
                pool(nc.gpsimd.tensor_tensor,
                     q[:, 2 * C:3 * C], xsc[:, 4 * C:5 * C],
                     xsc[:, 5 * C:6 * C], op=ADD)
                pool(nc.gpsimd.tensor_tensor,
                     h[:, 0:C], q[:, 0:C], q[:, C:2 * C], op=ADD)
                evac_mults_g(bt, ps, 2, xsc)
                pool(nc.gpsimd.tensor_tensor,
                     q[:, 3 * C:4 * C], xsc[:, 6 * C:7 * C],
                     xsc[:, 7 * C:8 * C], op=ADD)
                pool(nc.gpsimd.tensor_tensor,
                     h[:, C:2 * C], q[:, 2 * C:3 * C], q[:, 3 * C:4 * C],
                     op=ADD)
                pool(nc.gpsimd.tensor_tensor,
                     th3[:], h[:, 0:C], h[:, C:2 * C], op=ADD)
                return evac_finish(bt, th3)

            # ---- mains bt1..7 + evacs ----
            for bt in range(1, BT):
                ps_bt[bt] = alloc_ps()
                if bt == 1:
                    for kt in range(KT):
                        for g in range(3):
                            mm(ps_bt[bt], bt, kt, g,
                               start=(kt == 0), stop=(kt == KT - 1))
                else:
                    for g in range(3):
                        for kt in range(KT):
                            mm(ps_bt[bt], bt, kt, g,
                               start=(kt == 0), stop=(kt == KT - 1))
                # evac of previous bt overlaps this bt's stream
                prev = bt - 1
                if prev == 0:
                    obn = evac_dve_tree(prev, ps_bt[prev])
                else:
                    obn = evac_pool_tree(prev, ps_bt[prev])
                eng = nc.sync if prev < 6 else nc.scalar
                eng.dma_start(out_d[prev * 128:(prev + 1) * 128, :], obn[:])

            obn = evac_pool_tree(BT - 1, ps_bt[BT - 1])
            nc.scalar.dma_start(out_d[(BT - 1) * 128:BT * 128, :], obn[:])

    nc.compile()
    return nc


_NC_CACHE = None


def _get_nc():
    global _NC_CACHE
    if _NC_CACHE is None:
        _NC_CACHE = build_nc()
    return _NC_CACHE


def _prep_in_maps(x, centers, widths, rule_params):
    import ml_dtypes

    x = np.asarray(x, np.float32)
    centers = np.asarray(centers, np.float32)
    widths = np.asarray(widths, np.float32)
    rule_params = np.asarray(rule_params, np.float32)

    # xabd[p, bt*9+i] = x[bt*128+p, i] for i<8; 1.0 at i=8
    xab = np.ones((128, BT, DX), np.float32)
    xab[:, :, :D] = x.reshape(BT, 128, D).transpose(1, 0, 2)
    xab = np.ascontiguousarray(xab.reshape(128, BT * DX))
    cb = np.broadcast_to(centers.reshape(1, DM), (128, DM))
    cw2n = np.broadcast_to((1.0 / (2.0 * widths * widths)).reshape(1, DM),
                           (128, DM))
    cbw = np.ascontiguousarray(
        np.concatenate([cb, cw2n], axis=1, dtype=np.float32))
    eye = np.eye(128, dtype=ml_dtypes.bfloat16)

    # rule_params rows r = rA*1024 + q*4 + j -> per core [p, kt, rA, c]
    # with row order rB' = j*256 + q, kt = rB' tile of 128.
    rp4 = rule_params.reshape(NRA, 256, M, C).transpose(0, 2, 1, 3)
    rp4 = rp4.reshape(NRA, NRB, C)

    in_maps = []
    for c in range(N_CORES):
        ra0 = c * RA_LOC
        idx = np.empty((RA_LOC, 3), np.int64)
        for r in range(RA_LOC):
            ra = ra0 + r
            idx[r] = [(ra >> 4) & 3, (ra >> 2) & 3, ra & 3]
        k = np.arange(3)
        cA = centers[k[None, :], idx]
        wtA = widths[k[None, :], idx]
        cA3 = np.broadcast_to(cA.reshape(1, RA_LOC * 3), (128, RA_LOC * 3))
        nwA2 = np.broadcast_to(
            (1.0 / (2.0 * wtA * wtA)).reshape(1, RA_LOC * 3),
            (128, RA_LOC * 3))
        small2 = np.ascontiguousarray(
            np.concatenate([cA3, nwA2], axis=1, dtype=np.float32))

        rp_c = rp4[ra0:ra0 + RA_LOC]                     # [8, 1024, 144]
        rp_c = rp_c.reshape(RA_LOC, KT, 128, C).transpose(2, 1, 0, 3)
        rp_c = rp_c.reshape(128, KT, SC).astype(ml_dtypes.bfloat16)

        im = {"xabd": xab, "cbw": cbw, "small2": small2, "eye": eye}
        for kt in range(KT):
            im[f"rp{kt}"] = np.ascontiguousarray(rp_c[:, kt])
        in_maps.append(im)
    return in_maps


def kernel(x, centers, widths, rule_params, _trace=False):
    nc = _get_nc()
    in_maps = _prep_in_maps(x, centers, widths, rule_params)
    res = run_bass_kernel_spmd(nc, in_maps, core_ids=list(range(N_CORES)),
                               trace=_trace)
    out = np.sum([np.asarray(res.results[c]["out"], np.float32)
                  for c in range(N_CORES)], axis=0)
    if _trace:
        kernel._last_exec_time_ns = res.exec_time_ns
        kernel._last_results = res
    return out


# revision 5
# speedup vs baseline: 1.3200x; 1.3200x over previous
"""ANFIS Trainium2 kernel (8 NeuronCores, Bass/Tile) — v6.

Math (reference):
  mfs[b,i,j] = exp(-(x[b,i]-centers[i,j])^2 / (2*widths[i,j]^2))   [1024,8,4]
  w[b,r]     = prod_i mfs[b,i,idx_i(r)]    r in [0, 4^8=65536), i0 slowest
  w        <- w / sum_r w
  out[b,n]   = sum_r w[b,r] * ([x[b],1] . rule_params[r,:,n])      [1024,16]

Structure: w = wA (x) wB with wA over dims 0..2 (64 vals, split 8 rA per
core) and wB over dims 3..7 (1024 vals); r = rA*1024 + rB.  Denominator
factorizes: sum_r w = prod_i (sum_j mfs[b,i,j]).

Per core:  psum[b, rA, i*16+n] = sum_rB wB[b,rB] rp[rA*1024+rB, i*16+n]
(bf16 matmuls, rB contracted on partitions, kt = 8 k-tiles), evacuated as
psum * G with G[b, rA*9+i] = wA[b,rA]/denom[b] * xb[b,i], tree-summed over
rA and strided-reduced over i.  Core partials summed on host.

v6 schedule notes (v4 @58.1us + DMA/tail fixes; v5's fused-w3s and Pool
trees reverted — DVE big-TT is 1.2ns/col and Pool TT has ~460ns fixed
cost, both lose):
  - rp is 8 per-kt DRAM params/tiles spread round-robin over the three
    DMA queues by first-use time: sync kt0/3/6, scalar kt1/4, gpsimd
    kt2/5/7.  Per-kt tiles also give matmuls precise DMA deps.  Head
    inputs split: xabd (x only, 36KB) FIRST on sync; cbw + small2
    (slimmed to 24KB: xA3 is now an AP view of xabd) + eye on scalar.
  - bt0 AND bt1 wB^T via PE identity-matmul transposes; bt1's
    transposes interleave after bt0's kt3 where the PE would stall on
    rp DMA anyway.  XBARs bt2..7 all on the sync queue (the sync
    engine just idle-waits for each w3s; outs bt0-5 queue behind, bt6/7
    go on scalar).
  - j-scales: bt0-2 on DVE, bt3-7 on ACT (as v4).
  - mains for bt2..7 run group-outer (g0 kt0..7, g1, g2) so each psum
    group closes 1/3 into its bt: the next bt's psum-pool reuse never
    waits, and the last bt's evac mostly overlaps its own stream.
  - last bt uses the group-local pair tree so only ~1.4us of DVE work
    trails the final matmul; its out DMA rides the idle scalar queue.
"""

import sys

sys.path.insert(0, "/opt/trn_rl_repo")

import numpy as np

import concourse.bacc as bacc
import concourse.tile as tile
import concourse.mybir as mybir
from concourse.ap import AP
from concourse.bass_utils import run_bass_kernel_spmd


F32 = mybir.dt.float32
BF16 = mybir.dt.bfloat16
MULT = mybir.AluOpType.mult
ADD = mybir.AluOpType.add
SUB = mybir.AluOpType.subtract
EXP = mybir.ActivationFunctionType.Exp
AXX = mybir.AxisListType.X

N_CORES = 8
B = 1024
BT = 8          # batch tiles of 128
D = 8           # input dims
DX = D + 1      # xb width (x plus ones column)
M = 4           # membership fns per dim
NO = 16         # outputs
C = DX * NO                 # 144
NRA = 64        # 4^3 (dims 0..2)
RA_LOC = NRA // N_CORES     # 8 local rA per core
NRB = 1024      # 4^5 (dims 3..7)
KT = 8          # rB partition tiles of 128
GROUPS = [(0, 3), (3, 3), (6, 2)]
SC = RA_LOC * C  # 1152
DM = D * M       # 32

N_WARM = 10             # dummy warm-up matmuls (256 cols each)

O_CB = 0
O_CW2N = O_CB + DM                # 32
NCBW = O_CW2N + DM                # 64
O_CA3 = 0
O_NWA2 = O_CA3 + RA_LOC * 3       # 24
NSM2 = O_NWA2 + RA_LOC * 3        # 48


def _v(t, off, dims):
    """Custom free-dim view of a [128, F] SBUF tile AP."""
    part = list(t.ap[0])
    return AP(
        tensor=t.tensor,
        offset=t.offset + off,
        ap=[part] + [[s, n] for (s, n) in dims],
    )


def build_nc():
    nc = bacc.Bacc("TRN2", target_bir_lowering=False, debug=False,
                   num_devices=N_CORES)

    xabd_d = nc.declare_dram_parameter("xabd", [128, BT * DX], F32,
                                       isOutput=False)
    cbw_d = nc.declare_dram_parameter("cbw", [128, NCBW], F32,
                                      isOutput=False)
    small2_d = nc.declare_dram_parameter("small2", [128, NSM2], F32,
                                         isOutput=False)
    eye_d = nc.declare_dram_parameter("eye", [128, 128], BF16, isOutput=False)
    rp_d = [nc.declare_dram_parameter(f"rp{kt}", [128, SC], BF16,
                                      isOutput=False) for kt in range(KT)]
    out_d = nc.declare_dram_parameter("out", [B, NO], F32, isOutput=True)

    with tile.TileContext(nc) as tc:
        with (
            tc.tile_pool(name="const", bufs=1) as cpool,
            tc.tile_pool(name="rp", bufs=1) as rppool,
            tc.tile_pool(name="wbt", bufs=1) as wbtpool,
            tc.tile_pool(name="work", bufs=2) as work,
            tc.tile_pool(name="w3s", bufs=3) as w3spool,
            tc.tile_pool(name="psD", bufs=1, space="PSUM") as psDp,
            tc.tile_pool(name="evac", bufs=3) as evpool,
            tc.tile_pool(name="ps0", bufs=2, space="PSUM") as ps0p,
            tc.tile_pool(name="ps1", bufs=2, space="PSUM") as ps1p,
            tc.tile_pool(name="ps2", bufs=2, space="PSUM") as ps2p,
        ):
            # ---- input tiles + DMA issue (order per queue matters) ----
            xab_t = cpool.tile([128, BT * DX], F32, tag="xabd")
            cbw = cpool.tile([128, NCBW], F32, tag="cbw")
            small2 = cpool.tile([128, NSM2], F32, tag="small2")
            eye = cpool.tile([128, 128], BF16, tag="eye")
            rp = [rppool.tile([128, SC], BF16, tag=f"rp{kt}",
                              name=f"rp{kt}")
                  for kt in range(KT)]
            zs = cpool.tile([128, 512], BF16, tag="zs")

            nc.sync.dma_start(xab_t[:], xabd_d[:])
            nc.scalar.dma_start(cbw[:], cbw_d[:])
            nc.scalar.dma_start(small2[:], small2_d[:])
            nc.scalar.dma_start(eye[:], eye_d[:])
            for kt in (0, 3, 6):
                nc.sync.dma_start(rp[kt][:], rp_d[kt][:])
            for kt in (1, 4):
                nc.scalar.dma_start(rp[kt][:], rp_d[kt][:])
            for kt in (2, 5, 7):
                nc.gpsimd.dma_start(rp[kt][:], rp_d[kt][:])

            xab = xab_t[:]
            cb = cbw[:, O_CB:O_CB + DM]
            cw2n = cbw[:, O_CW2N:O_CW2N + DM]
            cA3 = small2[:, O_CA3:O_CA3 + RA_LOC * 3]
            nwA2 = small2[:, O_NWA2:O_NWA2 + RA_LOC * 3]

            # ---- PE warm-up: zero tile (DVE memset, no deps) + dummies ----
            nc.vector.memset(zs[:], 0)
            psD = [psDp.tile([128, 512], F32, tag="psD0", name="psD0"),
                   psDp.tile([128, 512], F32, tag="psD1", name="psD1")]
            for i in range(N_WARM):
                nc.tensor.matmul(psD[i % 2][:, 0:256], zs[:, 0:128],
                                 zs[:, 0:256], start=True, stop=True)

            # DVE stage chain: force scheduler to respect emission order
            last_dve = [None]

            def dve(op_fn, *args, **kwargs):
                i = op_fn(*args, **kwargs)
                if last_dve[0] is not None:
                    tile.add_dep_helper(i.ins, last_dve[0].ins, sync=False,
                                        reason="dve stage order")
                last_dve[0] = i
                return i

            # bt0 membership chain in its own small tiles (clean DMA dep)
            mfs0 = cpool.tile([128, DM], F32, tag="mfs0")
            mfsR = cpool.tile([128, (BT - 1) * DM], F32, tag="mfsR")

            def mfs_chain(mfst, nbt, xoff, tg):
                dift = work.tile([128, nbt * DM], F32, tag="dif" + tg)
                d2t = work.tile([128, nbt * DM], F32, tag="d2" + tg)
                d2st = work.tile([128, nbt * DM], F32, tag="d2s" + tg)
                dve(nc.vector.tensor_tensor,
                    _v(dift[:], 0, [(DM, nbt), (M, D), (1, M)]),
                    _v(xab, xoff, [(DX, nbt), (1, D), (0, M)]),
                    _v(cb, 0, [(0, nbt), (M, D), (1, M)]),
                    op=SUB)
                dve(nc.vector.tensor_tensor,
                    d2t[:], dift[:], dift[:], op=MULT)
                dve(nc.vector.tensor_tensor,
                    _v(d2st[:], 0, [(DM, nbt), (1, DM)]),
                    _v(d2t[:], 0, [(DM, nbt), (1, DM)]),
                    _v(cw2n, 0, [(0, nbt), (1, DM)]), op=MULT)
                nc.scalar.activation(mfst[:], d2st[:], EXP, scale=-1.0)

            # mfs column views per bt (bt0 in mfs0, rest in mfsR)
            def mfs_at(bt, off):
                if bt == 0:
                    return mfs0, off
                return mfsR, (bt - 1) * DM + off

            w34 = work.tile([128, BT * 16], BF16, tag="w34")
            w56 = work.tile([128, BT * 16], BF16, tag="w56")
            w3456 = cpool.tile([128, BT * 256], BF16, tag="w3456")

            def w_chain(off, nbt):
                mfst, moff = mfs_at(off, 0)
                dve(nc.vector.tensor_tensor,
                    _v(w34[:], off * 16, [(16, nbt), (M, M), (1, M)]),
                    _v(mfst[:], moff + 3 * M, [(DM, nbt), (1, M), (0, M)]),
                    _v(mfst[:], moff + 4 * M, [(DM, nbt), (0, M), (1, M)]),
                    op=MULT)
                dve(nc.vector.tensor_tensor,
                    _v(w56[:], off * 16, [(16, nbt), (M, M), (1, M)]),
                    _v(mfst[:], moff + 5 * M, [(DM, nbt), (1, M), (0, M)]),
                    _v(mfst[:], moff + 6 * M, [(DM, nbt), (0, M), (1, M)]),
                    op=MULT)
                dve(nc.vector.tensor_tensor,
                    _v(w3456[:], off * 256, [(256, nbt), (16, 16), (1, 16)]),
                    _v(w34[:], off * 16, [(16, nbt), (1, 16), (0, 16)]),
                    _v(w56[:], off * 16, [(16, nbt), (0, 16), (1, 16)]),
                    op=MULT)

            wbt = wbtpool.tile([128, KT * B], BF16, tag="wbt")

            def jscales(bt, on_dve):
                w3sall = w3spool.tile([128, 1024], BF16, tag="w3s",
                                      name="w3sall")
                mfst, moff = mfs_at(bt, 7 * M)
                for j in range(M):
                    dst = w3sall[:, j * 256:(j + 1) * 256]
                    src = w3456[:, bt * 256:(bt + 1) * 256]
                    sc = mfst[:, moff + j: moff + j + 1]
                    if on_dve:
                        dve(nc.vector.tensor_scalar_mul, dst, src, sc)
                    else:
                        nc.scalar.mul(dst, src, sc)
                return w3sall

            def pe_transpose(w3sb, bt):
                """wB^T for bt via PE identity matmuls + ACT psum copies."""
                for j in range(M):
                    for qh in range(2):
                        kt = 2 * j + qh
                        m, t = kt // 4, kt % 4
                        nc.tensor.matmul(
                            psD[m][:, t * 128:(t + 1) * 128],
                            w3sb[:, kt * 128:(kt + 1) * 128], eye[:],
                            start=True, stop=True)
                # split so kt0's slice lands first
                nc.scalar.copy(_v(wbt[:], bt * 128, [(B, 1), (1, 128)]),
                               psD[0][:, 0:128])
                nc.scalar.copy(_v(wbt[:], B + bt * 128, [(B, 3), (1, 128)]),
                               psD[0][:, 128:512])
                nc.scalar.copy(_v(wbt[:], 4 * B + bt * 128,
                                  [(B, 4), (1, 128)]),
                               psD[1][:])

            # ---- matmul emit helpers ----
            def mm(ps, bt, kt, g, start, stop):
                r0, nr = GROUPS[g]
                nc.tensor.matmul(
                    ps[g][:],
                    wbt[:, kt * B + bt * 128: kt * B + (bt + 1) * 128],
                    _v(rp[kt][:], r0 * C, [(C, nr), (1, C)]),
                    start=start, stop=stop)

            def alloc_ps():
                return [
                    ps0p.tile([128, GROUPS[0][1] * C], F32, tag="ps0",
                              name="ps0"),
                    ps1p.tile([128, GROUPS[1][1] * C], F32, tag="ps1",
                              name="ps1"),
                    ps2p.tile([128, GROUPS[2][1] * C], F32, tag="ps2",
                              name="ps2")]

            # ---- S1: bt0 chain -> w3s0 -> PE transpose -> bt0 kt0-3 ----
            mfs_chain(mfs0, 1, 0, "0")
            w_chain(0, 1)
            w3s0 = jscales(0, on_dve=True)
            pe_transpose(w3s0, 0)

            ps_bt = [None] * BT
            ps_bt[0] = alloc_ps()
            for kt in range(4):
                for g in range(3):
                    mm(ps_bt[0], 0, kt, g, start=(kt == 0), stop=False)

            # ---- S2: bulk chain; bt1 -> PE transpose mid-bt0 ----
            mfs_chain(mfsR, BT - 1, DX, "R")
            w_chain(1, 1)
            w3s1 = jscales(1, on_dve=True)
            pe_transpose(w3s1, 1)
            for kt in range(4, KT):
                for g in range(3):
                    mm(ps_bt[0], 0, kt, g, start=False, stop=(kt == KT - 1))

            # ---- S3: w3s for bt2 (DVE) + bt3-7 (ACT); XBARs on sync ----
            w_chain(2, BT - 2)
            w3s2 = jscales(2, on_dve=True)
            nc.sync.dma_start_transpose(
                _v(wbt[:], 2 * 128, [(B, KT), (1, 128)]), w3s2[:])
            for bt in range(3, BT):
                w3sb = jscales(bt, on_dve=False)
                nc.sync.dma_start_transpose(
                    _v(wbt[:], bt * 128, [(B, KT), (1, 128)]), w3sb[:])

            # ---- S4: wA chain -> G = wA*xb (UNNORMALIZED; 1/denom is
            #      applied per-bt to the reduced [128,16] output) ----
            NA = BT * RA_LOC * 3  # 192
            xA3v = _v(xab, 0, [(DX, BT), (0, RA_LOC), (1, 3)])
            dA = work.tile([128, NA], F32, tag="dA")
            dve(nc.vector.tensor_tensor,
                dA[:], xA3v, _v(cA3, 0, [(0, BT), (1, RA_LOC * 3)]), op=SUB)
            d2A = work.tile([128, NA], F32, tag="d2A")
            dve(nc.vector.tensor_tensor, d2A[:], dA[:], dA[:], op=MULT)
            d2sA = work.tile([128, NA], F32, tag="d2sA")
            dve(nc.vector.tensor_tensor,
                d2sA[:], d2A[:], _v(nwA2, 0, [(0, BT), (1, RA_LOC * 3)]),
                op=MULT)
            eA = work.tile([128, BT * RA_LOC], F32, tag="eA")
            dve(nc.vector.reduce_sum,
                eA[:], _v(d2sA[:], 0, [(3, BT * RA_LOC), (1, 3)]), axis=AXX)
            wA = cpool.tile([128, BT * RA_LOC], F32, tag="wA")
            nc.scalar.activation(wA[:], eA[:], EXP, scale=-1.0)

            Gall = cpool.tile([128, BT * RA_LOC * DX], F32, tag="Gall")
            dve(nc.vector.tensor_tensor,
                Gall[:],
                _v(wA[:], 0, [(RA_LOC, BT), (1, RA_LOC), (0, DX)]),
                _v(xab, 0, [(DX, BT), (0, RA_LOC), (1, DX)]), op=MULT)

            invd = cpool.tile([128, BT], F32, tag="invd")

            def denoms():
                s = work.tile([128, BT * D], F32, tag="s")
                dve(nc.vector.reduce_sum,
                    s[:, 0:D], _v(mfs0[:], 0, [(M, D), (1, M)]), axis=AXX)
                dve(nc.vector.reduce_sum,
                    s[:, D:BT * D],
                    _v(mfsR[:], 0, [(M, (BT - 1) * D), (1, M)]), axis=AXX)
                p1 = work.tile([128, BT * 4], F32, tag="p1")
                dve(nc.vector.tensor_tensor,
                    p1[:], _v(s[:], 0, [(D, BT), (1, 4)]),
                    _v(s[:], 4, [(D, BT), (1, 4)]), op=MULT)
                p2 = work.tile([128, BT * 2], F32, tag="p2")
                dve(nc.vector.tensor_tensor,
                    p2[:], _v(p1[:], 0, [(4, BT), (1, 2)]),
                    _v(p1[:], 2, [(4, BT), (1, 2)]), op=MULT)
                p3 = work.tile([128, BT], F32, tag="p3")
                dve(nc.vector.tensor_tensor,
                    p3[:], _v(p2[:], 0, [(2, BT)]), _v(p2[:], 1, [(2, BT)]),
                    op=MULT)
                dve(nc.vector.reciprocal, invd[:], p3[:])

            # ---- evac pieces ----
            obn_all = cpool.tile([128, BT * NO], F32, tag="obn_all")

            def evac_mults_g(bt, ps, g, xsc):
                r0, nr = GROUPS[g]
                dve(nc.vector.tensor_tensor,
                    xsc[:, r0 * C:(r0 + nr) * C], ps[g][:],
                    _v(Gall[:], bt * RA_LOC * DX + r0 * DX,
                       [(DX, nr), (1, DX), (0, NO)]),
                    op=MULT)

            def evac_finish(bt, th3):
                ob = evpool.tile([128, NO], F32, tag="ob")
                dve(nc.vector.reduce_sum,
                    ob[:], _v(th3[:], 0, [(1, NO), (NO, DX)]), axis=AXX)
                obn = obn_all[:, bt * NO:(bt + 1) * NO]
                dve(nc.vector.tensor_scalar_mul,
                    obn, ob[:], invd[:, bt:bt + 1])
                return obn

            def evac_tree(bt, ps, last):
                xsc = evpool.tile([128, SC], BF16, tag="xsc")
                th3 = evpool.tile([128, C], BF16, tag="th3")
                if not last:
                    for g in range(3):
                        evac_mults_g(bt, ps, g, xsc)
                    th = evpool.tile([128, 4 * C], BF16, tag="th")
                    dve(nc.vector.tensor_tensor,
                        th[:], xsc[:, 0:4 * C], xsc[:, 4 * C:8 * C], op=ADD)
                    th2 = evpool.tile([128, 2 * C], BF16, tag="th2")
                    dve(nc.vector.tensor_tensor,
                        th2[:], th[:, 0:2 * C], th[:, 2 * C:4 * C], op=ADD)
                    dve(nc.vector.tensor_tensor,
                        th3[:], th2[:, 0:C], th2[:, C:2 * C], op=ADD)
                else:
                    # group-local pair tree: tail after the g2 mult is only
                    # q3 + h1 + th3 + reduce + scale (~1.4us)
                    q = evpool.tile([128, 4 * C], BF16, tag="th")
                    h = evpool.tile([128, 2 * C], BF16, tag="th2")
                    evac_mults_g(bt, ps, 0, xsc)
                    dve(nc.vector.tensor_tensor,
                        q[:, 0:C], xsc[:, 0:C], xsc[:, C:2 * C], op=ADD)
                    evac_mults_g(bt, ps, 1, xsc)
                    dve(nc.vector.tensor_tensor,
                        q[:, C:2 * C], xsc[:, 2 * C:3 * C],
                        xsc[:, 3 * C:4 * C], op=ADD)
                    dve(nc.vector.tensor_tensor,
                        q[:, 2 * C:3 * C], xsc[:, 4 * C:5 * C],
                        xsc[:, 5 * C:6 * C], op=ADD)
                    dve(nc.vector.tensor_tensor,
                        h[:, 0:C], q[:, 0:C], q[:, C:2 * C], op=ADD)
                    evac_mults_g(bt, ps, 2, xsc)
                    dve(nc.vector.tensor_tensor,
                        q[:, 3 * C:4 * C], xsc[:, 6 * C:7 * C],
                        xsc[:, 7 * C:8 * C], op=ADD)
                    dve(nc.vector.tensor_tensor,
                        h[:, C:2 * C], q[:, 2 * C:3 * C], q[:, 3 * C:4 * C],
                        op=ADD)
                    dve(nc.vector.tensor_tensor,
                        th3[:], h[:, 0:C], h[:, C:2 * C], op=ADD)
                return evac_finish(bt, th3)

            # ---- S6: mains bt1..7 + evacs ----
            for bt in range(1, BT):
                ps_bt[bt] = alloc_ps()
                if bt == 1:
                    for kt in range(KT):
                        for g in range(3):
                            mm(ps_bt[bt], bt, kt, g,
                               start=(kt == 0), stop=(kt == KT - 1))
                else:
                    for g in range(3):
                        for kt in range(KT):
                            mm(ps_bt[bt], bt, kt, g,
                               start=(kt == 0), stop=(kt == KT - 1))
                prev = bt - 1
                if prev == 0:
                    denoms()
                obn = evac_tree(prev, ps_bt[prev], last=False)
                eng = nc.sync if prev < 6 else nc.scalar
                eng.dma_start(out_d[prev * 128:(prev + 1) * 128, :], obn)

            obn = evac_tree(BT - 1, ps_bt[BT - 1], last=True)
            nc.scalar.dma_start(out_d[(BT - 1) * 128:BT * 128, :], obn)

    nc.compile()
    return nc


_NC_CACHE = None


def _get_nc():
    global _NC_CACHE
    if _NC_CACHE is None:
        _NC_CACHE = build_nc()
    return _NC_CACHE


def _prep_in_maps(x, centers, widths, rule_params):
    import ml_dtypes

    x = np.asarray(x, np.float32)
    centers = np.asarray(centers, np.float32)
    widths = np.asarray(widths, np.float32)
    rule_params = np.asarray(rule_params, np.float32)

    # xabd[p, bt*9+i] = x[bt*128+p, i] for i<8; 1.0 at i=8
    xab = np.ones((128, BT, DX), np.float32)
    xab[:, :, :D] = x.reshape(BT, 128, D).transpose(1, 0, 2)
    xab = np.ascontiguousarray(xab.reshape(128, BT * DX))
    cb = np.broadcast_to(centers.reshape(1, DM), (128, DM))
    cw2n = np.broadcast_to((1.0 / (2.0 * widths * widths)).reshape(1, DM),
                           (128, DM))
    cbw = np.ascontiguousarray(
        np.concatenate([cb, cw2n], axis=1, dtype=np.float32))
    eye = np.eye(128, dtype=ml_dtypes.bfloat16)

    # rule_params rows r = rA*1024 + q*4 + j -> per core [p, kt, rA, c]
    # with row order rB' = j*256 + q, kt = rB' tile of 128.
    rp4 = rule_params.reshape(NRA, 256, M, C).transpose(0, 2, 1, 3)
    rp4 = rp4.reshape(NRA, NRB, C)

    in_maps = []
    for c in range(N_CORES):
        ra0 = c * RA_LOC
        idx = np.empty((RA_LOC, 3), np.int64)
        for r in range(RA_LOC):
            ra = ra0 + r
            idx[r] = [(ra >> 4) & 3, (ra >> 2) & 3, ra & 3]
        k = np.arange(3)
        cA = centers[k[None, :], idx]
        wtA = widths[k[None, :], idx]
        cA3 = np.broadcast_to(cA.reshape(1, RA_LOC * 3), (128, RA_LOC * 3))
        nwA2 = np.broadcast_to(
            (1.0 / (2.0 * wtA * wtA)).reshape(1, RA_LOC * 3),
            (128, RA_LOC * 3))
        small2 = np.ascontiguousarray(
            np.concatenate([cA3, nwA2], axis=1, dtype=np.float32))

        rp_c = rp4[ra0:ra0 + RA_LOC]                     # [8, 1024, 144]
        rp_c = rp_c.reshape(RA_LOC, KT, 128, C).transpose(2, 1, 0, 3)
        rp_c = rp_c.reshape(128, KT, SC).astype(ml_dtypes.bfloat16)

        im = {"xabd": xab, "cbw": cbw, "small2": small2, "eye": eye}
        for kt in range(KT):
            im[f"rp{kt}"] = np.ascontiguousarray(rp_c[:, kt])
        in_maps.append(im)
    return in_maps


def kernel(x, centers, widths, rule_params, _trace=False):
    nc = _get_nc()
    in_maps = _prep_in_maps(x, centers, widths, rule_params)
    res = run_bass_kernel_spmd(nc, in_maps, core_ids=list(range(N_CORES)),
                               trace=_trace)
    out = np.sum([np.asarray(res.results[c]["out"], np.float32)
                  for c in range(N_CORES)], axis=0)
    if _trace:
        kernel._last_exec_time_ns = res.exec_time_ns
        kernel._last_results = res
    return out


# revision 9
# speedup vs baseline: 1.4226x; 1.0777x over previous
"""ANFIS Trainium2 kernel (8 NeuronCores, Bass/Tile) — v6.

Math (reference):
  mfs[b,i,j] = exp(-(x[b,i]-centers[i,j])^2 / (2*widths[i,j]^2))   [1024,8,4]
  w[b,r]     = prod_i mfs[b,i,idx_i(r)]    r in [0, 4^8=65536), i0 slowest
  w        <- w / sum_r w
  out[b,n]   = sum_r w[b,r] * ([x[b],1] . rule_params[r,:,n])      [1024,16]

Structure: w = wA (x) wB with wA over dims 0..2 (64 vals, split 8 rA per
core) and wB over dims 3..7 (1024 vals); r = rA*1024 + rB.  Denominator
factorizes: sum_r w = prod_i (sum_j mfs[b,i,j]).

Per core:  psum[b, rA, i*16+n] = sum_rB wB[b,rB] rp[rA*1024+rB, i*16+n]
(bf16 matmuls, rB contracted on partitions, kt = 8 k-tiles), evacuated as
psum * G with G[b, rA*9+i] = wA[b,rA]/denom[b] * xb[b,i], tree-summed over
rA and strided-reduced over i.  Core partials summed on host.

v6 schedule notes (v4 @58.1us + DMA/tail fixes; v5's fused-w3s and Pool
trees reverted — DVE big-TT is 1.2ns/col and Pool TT has ~460ns fixed
cost, both lose):
  - rp is 8 per-kt DRAM params/tiles spread round-robin over the three
    DMA queues by first-use time: sync kt0/3/6, scalar kt1/4, gpsimd
    kt2/5/7.  Per-kt tiles also give matmuls precise DMA deps.  Head
    inputs split: xabd (x only, 36KB) FIRST on sync; cbw + small2
    (slimmed to 24KB: xA3 is now an AP view of xabd) + eye on scalar.
  - bt0 AND bt1 wB^T via PE identity-matmul transposes; bt1's
    transposes interleave after bt0's kt3 where the PE would stall on
    rp DMA anyway.  XBARs bt2..7 all on the sync queue (the sync
    engine just idle-waits for each w3s; outs bt0-5 queue behind, bt6/7
    go on scalar).
  - j-scales: bt0-2 on DVE, bt3-7 on ACT (as v4).
  - mains for bt2..7 run group-outer (g0 kt0..7, g1, g2) so each psum
    group closes 1/3 into its bt: the next bt's psum-pool reuse never
    waits, and the last bt's evac mostly overlaps its own stream.
  - last bt uses the group-local pair tree so only ~1.4us of DVE work
    trails the final matmul; its out DMA rides the idle scalar queue.
"""

import sys

sys.path.insert(0, "/opt/trn_rl_repo")

import numpy as np

import concourse.bacc as bacc
import concourse.tile as tile
import concourse.mybir as mybir
from concourse.ap import AP
from concourse.bass_utils import run_bass_kernel_spmd


F32 = mybir.dt.float32
BF16 = mybir.dt.bfloat16
MULT = mybir.AluOpType.mult
ADD = mybir.AluOpType.add
SUB = mybir.AluOpType.subtract
EXP = mybir.ActivationFunctionType.Exp
AXX = mybir.AxisListType.X

N_CORES = 8
B = 1024
BT = 8          # batch tiles of 128
D = 8           # input dims
DX = D + 1      # xb width (x plus ones column)
M = 4           # membership fns per dim
NO = 16         # outputs
C = DX * NO                 # 144
NRA = 64        # 4^3 (dims 0..2)
RA_LOC = NRA // N_CORES     # 8 local rA per core
NRB = 1024      # 4^5 (dims 3..7)
KT = 8          # rB partition tiles of 128
GROUPS = [(0, 3), (3, 3), (6, 2)]
SC = RA_LOC * C  # 1152
DM = D * M       # 32

N_WARM = 10             # dummy warm-up matmuls (256 cols each)

O_CB = 0
O_CW2N = O_CB + DM                # 32
NCBW = O_CW2N + DM                # 64
O_CA3 = 0
O_NWA2 = O_CA3 + RA_LOC * 3       # 24
NSM2 = O_NWA2 + RA_LOC * 3        # 48


def _v(t, off, dims):
    """Custom free-dim view of a [128, F] SBUF tile AP."""
    part = list(t.ap[0])
    return AP(
        tensor=t.tensor,
        offset=t.offset + off,
        ap=[part] + [[s, n] for (s, n) in dims],
    )


def build_nc():
    nc = bacc.Bacc("TRN2", target_bir_lowering=False, debug=False,
                   num_devices=N_CORES)

    xabd_d = nc.declare_dram_parameter("xabd", [128, BT * DX], F32,
                                       isOutput=False)
    cbw_d = nc.declare_dram_parameter("cbw", [128, NCBW], F32,
                                      isOutput=False)
    small2_d = nc.declare_dram_parameter("small2", [128, NSM2], F32,
                                         isOutput=False)
    eye_d = nc.declare_dram_parameter("eye", [128, 128], BF16, isOutput=False)
    rp_d = [nc.declare_dram_parameter(f"rp{kt}", [128, SC], BF16,
                                      isOutput=False) for kt in range(KT)]
    out_d = nc.declare_dram_parameter("out", [B, NO], F32, isOutput=True)

    with tile.TileContext(nc) as tc:
        with (
            tc.tile_pool(name="const", bufs=1) as cpool,
            tc.tile_pool(name="rp", bufs=1) as rppool,
            tc.tile_pool(name="wbt", bufs=1) as wbtpool,
            tc.tile_pool(name="work", bufs=2) as work,
            tc.tile_pool(name="w3s", bufs=3) as w3spool,
            tc.tile_pool(name="psD", bufs=1, space="PSUM") as psDp,
            tc.tile_pool(name="evac", bufs=3) as evpool,
            tc.tile_pool(name="ps0", bufs=2, space="PSUM") as ps0p,
            tc.tile_pool(name="ps1", bufs=2, space="PSUM") as ps1p,
            tc.tile_pool(name="ps2", bufs=2, space="PSUM") as ps2p,
        ):
            # ---- input tiles + DMA issue (order per queue matters) ----
            xab_t = cpool.tile([128, BT * DX], F32, tag="xabd")
            cbw = cpool.tile([128, NCBW], F32, tag="cbw")
            small2 = cpool.tile([128, NSM2], F32, tag="small2")
            eye = cpool.tile([128, 128], BF16, tag="eye")
            rp = [rppool.tile([128, SC], BF16, tag=f"rp{kt}",
                              name=f"rp{kt}")
                  for kt in range(KT)]
            zs = cpool.tile([128, 512], BF16, tag="zs")

            # sync: head-of-chain inputs then its rp share; scalar: eye
            # FIRST (gates the bt0 PE transpose), its rp share lands last
            # under HBM contention so it gets the latest-consumed chunks.
            nc.sync.dma_start(xab_t[:], xabd_d[:])
            nc.sync.dma_start(cbw[:], cbw_d[:])
            nc.scalar.dma_start(eye[:], eye_d[:])
            nc.scalar.dma_start(small2[:], small2_d[:])
            for kt in (0, 3, 6):
                nc.sync.dma_start(rp[kt][:], rp_d[kt][:])
            for kt in (1, 4):
                nc.scalar.dma_start(rp[kt][:], rp_d[kt][:])
            for kt in (2, 5, 7):
                nc.gpsimd.dma_start(rp[kt][:], rp_d[kt][:])

            xab = xab_t[:]
            cb = cbw[:, O_CB:O_CB + DM]
            cw2n = cbw[:, O_CW2N:O_CW2N + DM]
            cA3 = small2[:, O_CA3:O_CA3 + RA_LOC * 3]
            nwA2 = small2[:, O_NWA2:O_NWA2 + RA_LOC * 3]

            # ---- PE warm-up: zero tile (DVE memset, no deps) + dummies ----
            nc.vector.memset(zs[:], 0)
            psD = [psDp.tile([128, 512], F32, tag="psD0", name="psD0"),
                   psDp.tile([128, 512], F32, tag="psD1", name="psD1")]
            for i in range(N_WARM):
                nc.tensor.matmul(psD[i % 2][:, 0:256], zs[:, 0:128],
                                 zs[:, 0:256], start=True, stop=True)

            # DVE stage chain: force scheduler to respect emission order
            last_dve = [None]

            def dve(op_fn, *args, **kwargs):
                i = op_fn(*args, **kwargs)
                if last_dve[0] is not None:
                    tile.add_dep_helper(i.ins, last_dve[0].ins, sync=False,
                                        reason="dve stage order")
                last_dve[0] = i
                return i

            # bt0 membership chain in its own small tiles (clean DMA dep)
            mfs0 = cpool.tile([128, DM], F32, tag="mfs0")
            mfsR = cpool.tile([128, (BT - 1) * DM], F32, tag="mfsR")

            def mfs_chain(mfst, nbt, xoff, tg):
                dift = work.tile([128, nbt * DM], F32, tag="dif" + tg)
                d2t = work.tile([128, nbt * DM], F32, tag="d2" + tg)
                d2st = work.tile([128, nbt * DM], F32, tag="d2s" + tg)
                dve(nc.vector.tensor_tensor,
                    _v(dift[:], 0, [(DM, nbt), (M, D), (1, M)]),
                    _v(xab, xoff, [(DX, nbt), (1, D), (0, M)]),
                    _v(cb, 0, [(0, nbt), (M, D), (1, M)]),
                    op=SUB)
                dve(nc.vector.tensor_tensor,
                    d2t[:], dift[:], dift[:], op=MULT)
                dve(nc.vector.tensor_tensor,
                    _v(d2st[:], 0, [(DM, nbt), (1, DM)]),
                    _v(d2t[:], 0, [(DM, nbt), (1, DM)]),
                    _v(cw2n, 0, [(0, nbt), (1, DM)]), op=MULT)
                nc.scalar.activation(mfst[:], d2st[:], EXP, scale=-1.0)

            # mfs column views per bt (bt0 in mfs0, rest in mfsR)
            def mfs_at(bt, off):
                if bt == 0:
                    return mfs0, off
                return mfsR, (bt - 1) * DM + off

            w34 = work.tile([128, BT * 16], BF16, tag="w34")
            w56 = work.tile([128, BT * 16], BF16, tag="w56")
            w3456 = cpool.tile([128, BT * 256], BF16, tag="w3456")

            def w_chain(off, nbt):
                mfst, moff = mfs_at(off, 0)
                dve(nc.vector.tensor_tensor,
                    _v(w34[:], off * 16, [(16, nbt), (M, M), (1, M)]),
                    _v(mfst[:], moff + 3 * M, [(DM, nbt), (1, M), (0, M)]),
                    _v(mfst[:], moff + 4 * M, [(DM, nbt), (0, M), (1, M)]),
                    op=MULT)
                dve(nc.vector.tensor_tensor,
                    _v(w56[:], off * 16, [(16, nbt), (M, M), (1, M)]),
                    _v(mfst[:], moff + 5 * M, [(DM, nbt), (1, M), (0, M)]),
                    _v(mfst[:], moff + 6 * M, [(DM, nbt), (0, M), (1, M)]),
                    op=MULT)
                dve(nc.vector.tensor_tensor,
                    _v(w3456[:], off * 256, [(256, nbt), (16, 16), (1, 16)]),
                    _v(w34[:], off * 16, [(16, nbt), (1, 16), (0, 16)]),
                    _v(w56[:], off * 16, [(16, nbt), (0, 16), (1, 16)]),
                    op=MULT)

            wbt = wbtpool.tile([128, KT * B], BF16, tag="wbt")

            def jscales(bt, on_dve):
                w3sall = w3spool.tile([128, 1024], BF16, tag="w3s",
                                      name="w3sall")
                mfst, moff = mfs_at(bt, 7 * M)
                for j in range(M):
                    dst = w3sall[:, j * 256:(j + 1) * 256]
                    src = w3456[:, bt * 256:(bt + 1) * 256]
                    sc = mfst[:, moff + j: moff + j + 1]
                    if on_dve:
                        dve(nc.vector.tensor_scalar_mul, dst, src, sc)
                    else:
                        nc.scalar.mul(dst, src, sc)
                return w3sall

            def pe_transpose(w3sb, bt):
                """wB^T for bt via PE identity matmuls + ACT psum copies."""
                for j in range(M):
                    for qh in range(2):
                        kt = 2 * j + qh
                        m, t = kt // 4, kt % 4
                        nc.tensor.matmul(
                            psD[m][:, t * 128:(t + 1) * 128],
                            w3sb[:, kt * 128:(kt + 1) * 128], eye[:],
                            start=True, stop=True)
                # split so kt0's slice lands first
                nc.scalar.copy(_v(wbt[:], bt * 128, [(B, 1), (1, 128)]),
                               psD[0][:, 0:128])
                nc.scalar.copy(_v(wbt[:], B + bt * 128, [(B, 3), (1, 128)]),
                               psD[0][:, 128:512])
                nc.scalar.copy(_v(wbt[:], 4 * B + bt * 128,
                                  [(B, 4), (1, 128)]),
                               psD[1][:])

            # ---- matmul emit helpers ----
            def mm(ps, bt, kt, g, start, stop):
                r0, nr = GROUPS[g]
                nc.tensor.matmul(
                    ps[g][:],
                    wbt[:, kt * B + bt * 128: kt * B + (bt + 1) * 128],
                    _v(rp[kt][:], r0 * C, [(C, nr), (1, C)]),
                    start=start, stop=stop)

            def alloc_ps():
                return [
                    ps0p.tile([128, GROUPS[0][1] * C], F32, tag="ps0",
                              name="ps0"),
                    ps1p.tile([128, GROUPS[1][1] * C], F32, tag="ps1",
                              name="ps1"),
                    ps2p.tile([128, GROUPS[2][1] * C], F32, tag="ps2",
                              name="ps2")]

            # ---- S1: membership chains (bt0 first, bulk right behind so
            #      its ACT exp isn't queued behind the psD0 copies) ----
            mfs_chain(mfs0, 1, 0, "0")
            mfs_chain(mfsR, BT - 1, DX, "R")

            # ---- S2: bt0 -> w3s0 -> PE transpose; bt0 mains consume kt
            #      chunks in expected DMA-landing order (psum accumulation
            #      order is free); bt1's PE transpose fills the idle slot
            #      while bt0 waits on the late rp chunks ----
            w_chain(0, 1)
            w3s0 = jscales(0, on_dve=True)
            pe_transpose(w3s0, 0)

            BT0_KT_A = (0, 2, 5, 3)
            BT0_KT_B = (7, 1, 6, 4)
            ps_bt = [None] * BT
            ps_bt[0] = alloc_ps()
            for kt in BT0_KT_A:
                for g in range(3):
                    mm(ps_bt[0], 0, kt, g, start=(kt == BT0_KT_A[0]),
                       stop=False)

            w_chain(1, 1)
            w3s1 = jscales(1, on_dve=True)
            pe_transpose(w3s1, 1)
            for kt in BT0_KT_B:
                for g in range(3):
                    mm(ps_bt[0], 0, kt, g, start=False,
                       stop=(kt == BT0_KT_B[-1]))

            # ---- S3: w3s for bt2 (DVE) + XBAR ----
            w_chain(2, BT - 2)
            w3s2 = jscales(2, on_dve=True)
            nc.sync.dma_start_transpose(
                _v(wbt[:], 2 * 128, [(B, KT), (1, 128)]), w3s2[:])

            # ---- S4: wA chain -> G = wA*xb (UNNORMALIZED; 1/denom is
            #      applied per-bt to the reduced [128,16] output) ----
            NA = BT * RA_LOC * 3  # 192
            xA3v = _v(xab, 0, [(DX, BT), (0, RA_LOC), (1, 3)])
            dA = work.tile([128, NA], F32, tag="dA")
            dve(nc.vector.tensor_tensor,
                dA[:], xA3v, _v(cA3, 0, [(0, BT), (1, RA_LOC * 3)]), op=SUB)
            d2A = work.tile([128, NA], F32, tag="d2A")
            dve(nc.vector.tensor_tensor, d2A[:], dA[:], dA[:], op=MULT)
            d2sA = work.tile([128, NA], F32, tag="d2sA")
            dve(nc.vector.tensor_tensor,
                d2sA[:], d2A[:], _v(nwA2, 0, [(0, BT), (1, RA_LOC * 3)]),
                op=MULT)
            eA = work.tile([128, BT * RA_LOC], F32, tag="eA")
            dve(nc.vector.reduce_sum,
                eA[:], _v(d2sA[:], 0, [(3, BT * RA_LOC), (1, 3)]), axis=AXX)
            wA = cpool.tile([128, BT * RA_LOC], F32, tag="wA")
            nc.scalar.activation(wA[:], eA[:], EXP, scale=-1.0)

            Gall = cpool.tile([128, BT * RA_LOC * DX], F32, tag="Gall")
            dve(nc.vector.tensor_tensor,
                Gall[:],
                _v(wA[:], 0, [(RA_LOC, BT), (1, RA_LOC), (0, DX)]),
                _v(xab, 0, [(DX, BT), (0, RA_LOC), (1, DX)]), op=MULT)

            invd = cpool.tile([128, BT], F32, tag="invd")

            def denoms():
                s = work.tile([128, BT * D], F32, tag="s")
                dve(nc.vector.reduce_sum,
                    s[:, 0:D], _v(mfs0[:], 0, [(M, D), (1, M)]), axis=AXX)
                dve(nc.vector.reduce_sum,
                    s[:, D:BT * D],
                    _v(mfsR[:], 0, [(M, (BT - 1) * D), (1, M)]), axis=AXX)
                p1 = work.tile([128, BT * 4], F32, tag="p1")
                dve(nc.vector.tensor_tensor,
                    p1[:], _v(s[:], 0, [(D, BT), (1, 4)]),
                    _v(s[:], 4, [(D, BT), (1, 4)]), op=MULT)
                p2 = work.tile([128, BT * 2], F32, tag="p2")
                dve(nc.vector.tensor_tensor,
                    p2[:], _v(p1[:], 0, [(4, BT), (1, 2)]),
                    _v(p1[:], 2, [(4, BT), (1, 2)]), op=MULT)
                p3 = work.tile([128, BT], F32, tag="p3")
                dve(nc.vector.tensor_tensor,
                    p3[:], _v(p2[:], 0, [(2, BT)]), _v(p2[:], 1, [(2, BT)]),
                    op=MULT)
                dve(nc.vector.reciprocal, invd[:], p3[:])

            denoms()

            # ---- S5: jscales bt3-7 on ACT (AFTER the wA exp so Gall
            #      never queues behind them) + XBARs on sync ----
            for bt in range(3, BT):
                w3sb = jscales(bt, on_dve=False)
                nc.sync.dma_start_transpose(
                    _v(wbt[:], bt * 128, [(B, KT), (1, 128)]), w3sb[:])

            # ---- evac pieces ----
            obn_all = cpool.tile([128, BT * NO], F32, tag="obn_all")

            def evac_mults_g(bt, ps, g, xsc):
                r0, nr = GROUPS[g]
                dve(nc.vector.tensor_tensor,
                    xsc[:, r0 * C:(r0 + nr) * C], ps[g][:],
                    _v(Gall[:], bt * RA_LOC * DX + r0 * DX,
                       [(DX, nr), (1, DX), (0, NO)]),
                    op=MULT)

            def evac_finish(bt, th3):
                ob = evpool.tile([128, NO], F32, tag="ob")
                dve(nc.vector.reduce_sum,
                    ob[:], _v(th3[:], 0, [(1, NO), (NO, DX)]), axis=AXX)
                obn = obn_all[:, bt * NO:(bt + 1) * NO]
                dve(nc.vector.tensor_scalar_mul,
                    obn, ob[:], invd[:, bt:bt + 1])
                return obn

            def evac_tree(bt, ps, last):
                xsc = evpool.tile([128, SC], BF16, tag="xsc")
                th3 = evpool.tile([128, C], BF16, tag="th3")
                if not last:
                    for g in range(3):
                        evac_mults_g(bt, ps, g, xsc)
                    th = evpool.tile([128, 4 * C], BF16, tag="th")
                    dve(nc.vector.tensor_tensor,
                        th[:], xsc[:, 0:4 * C], xsc[:, 4 * C:8 * C], op=ADD)
                    th2 = evpool.tile([128, 2 * C], BF16, tag="th2")
                    dve(nc.vector.tensor_tensor,
                        th2[:], th[:, 0:2 * C], th[:, 2 * C:4 * C], op=ADD)
                    dve(nc.vector.tensor_tensor,
                        th3[:], th2[:, 0:C], th2[:, C:2 * C], op=ADD)
                else:
                    # group-local pair tree: tail after the g2 mult is only
                    # q3 + h1 + th3 + reduce + scale (~1.4us)
                    q = evpool.tile([128, 4 * C], BF16, tag="th")
                    h = evpool.tile([128, 2 * C], BF16, tag="th2")
                    evac_mults_g(bt, ps, 0, xsc)
                    dve(nc.vector.tensor_tensor,
                        q[:, 0:C], xsc[:, 0:C], xsc[:, C:2 * C], op=ADD)
                    evac_mults_g(bt, ps, 1, xsc)
                    dve(nc.vector.tensor_tensor,
                        q[:, C:2 * C], xsc[:, 2 * C:3 * C],
                        xsc[:, 3 * C:4 * C], op=ADD)
                    dve(nc.vector.tensor_tensor,
                        q[:, 2 * C:3 * C], xsc[:, 4 * C:5 * C],
                        xsc[:, 5 * C:6 * C], op=ADD)
                    dve(nc.vector.tensor_tensor,
                        h[:, 0:C], q[:, 0:C], q[:, C:2 * C], op=ADD)
                    evac_mults_g(bt, ps, 2, xsc)
                    dve(nc.vector.tensor_tensor,
                        q[:, 3 * C:4 * C], xsc[:, 6 * C:7 * C],
                        xsc[:, 7 * C:8 * C], op=ADD)
                    dve(nc.vector.tensor_tensor,
                        h[:, C:2 * C], q[:, 2 * C:3 * C], q[:, 3 * C:4 * C],
                        op=ADD)
                    dve(nc.vector.tensor_tensor,
                        th3[:], h[:, 0:C], h[:, C:2 * C], op=ADD)
                return evac_finish(bt, th3)

            # ---- S6: mains bt1..7 + evacs ----
            for bt in range(1, BT):
                ps_bt[bt] = alloc_ps()
                if bt == 1:
                    for kt in range(KT):
                        for g in range(3):
                            mm(ps_bt[bt], bt, kt, g,
                               start=(kt == 0), stop=(kt == KT - 1))
                else:
                    for g in range(3):
                        for kt in range(KT):
                            mm(ps_bt[bt], bt, kt, g,
                               start=(kt == 0), stop=(kt == KT - 1))
                prev = bt - 1
                obn = evac_tree(prev, ps_bt[prev], last=False)
                eng = nc.sync if prev < 6 else nc.scalar
                eng.dma_start(out_d[prev * 128:(prev + 1) * 128, :], obn)

            obn = evac_tree(BT - 1, ps_bt[BT - 1], last=True)
            nc.scalar.dma_start(out_d[(BT - 1) * 128:BT * 128, :], obn)

    nc.compile()
    return nc


_NC_CACHE = None


def _get_nc():
    global _NC_CACHE
    if _NC_CACHE is None:
        _NC_CACHE = build_nc()
    return _NC_CACHE


def _prep_in_maps(x, centers, widths, rule_params):
    import ml_dtypes

    x = np.asarray(x, np.float32)
    centers = np.asarray(centers, np.float32)
    widths = np.asarray(widths, np.float32)
    rule_params = np.asarray(rule_params, np.float32)

    # xabd[p, bt*9+i] = x[bt*128+p, i] for i<8; 1.0 at i=8
    xab = np.ones((128, BT, DX), np.float32)
    xab[:, :, :D] = x.reshape(BT, 128, D).transpose(1, 0, 2)
    xab = np.ascontiguousarray(xab.reshape(128, BT * DX))
    cb = np.broadcast_to(centers.reshape(1, DM), (128, DM))
    cw2n = np.broadcast_to((1.0 / (2.0 * widths * widths)).reshape(1, DM),
                           (128, DM))
    cbw = np.ascontiguousarray(
        np.concatenate([cb, cw2n], axis=1, dtype=np.float32))
    eye = np.eye(128, dtype=ml_dtypes.bfloat16)

    # rule_params rows r = rA*1024 + q*4 + j -> per core [p, kt, rA, c]
    # with row order rB' = j*256 + q, kt = rB' tile of 128.
    rp4 = rule_params.reshape(NRA, 256, M, C).transpose(0, 2, 1, 3)
    rp4 = rp4.reshape(NRA, NRB, C)

    in_maps = []
    for c in range(N_CORES):
        ra0 = c * RA_LOC
        idx = np.empty((RA_LOC, 3), np.int64)
        for r in range(RA_LOC):
            ra = ra0 + r
            idx[r] = [(ra >> 4) & 3, (ra >> 2) & 3, ra & 3]
        k = np.arange(3)
        cA = centers[k[None, :], idx]
        wtA = widths[k[None, :], idx]
        cA3 = np.broadcast_to(cA.reshape(1, RA_LOC * 3), (128, RA_LOC * 3))
        nwA2 = np.broadcast_to(
            (1.0 / (2.0 * wtA * wtA)).reshape(1, RA_LOC * 3),
            (128, RA_LOC * 3))
        small2 = np.ascontiguousarray(
            np.concatenate([cA3, nwA2], axis=1, dtype=np.float32))

        rp_c = rp4[ra0:ra0 + RA_LOC]                     # [8, 1024, 144]
        rp_c = rp_c.reshape(RA_LOC, KT, 128, C).transpose(2, 1, 0, 3)
        rp_c = rp_c.reshape(128, KT, SC).astype(ml_dtypes.bfloat16)

        im = {"xabd": xab, "cbw": cbw, "small2": small2, "eye": eye}
        for kt in range(KT):
            im[f"rp{kt}"] = np.ascontiguousarray(rp_c[:, kt])
        in_maps.append(im)
    return in_maps


def kernel(x, centers, widths, rule_params, _trace=False):
    nc = _get_nc()
    in_maps = _prep_in_maps(x, centers, widths, rule_params)
    res = run_bass_kernel_spmd(nc, in_maps, core_ids=list(range(N_CORES)),
                               trace=_trace)
    out = np.sum([np.asarray(res.results[c]["out"], np.float32)
                  for c in range(N_CORES)], axis=0)
    if _trace:
        kernel._last_exec_time_ns = res.exec_time_ns
        kernel._last_results = res
    return out
